# revision 24
# baseline (speedup 1.0000x reference)
"""Trainium2 Bass kernel for the ESN (echo state network) forward scan.

  x_{t+1} = (1-a) x_t + a tanh(u_t + x_t @ W),  a = 0.5
  U = einsum('bit,in->tbn', Input, W_in);  out X[b,n,t] = x_{t+1}[b,n]

Sharding: data-parallel over batch (B=64 -> 8 cores x 8 batches).
W, W_in replicated; no collectives. Each core runs the full T=2000 scan
for its 8 batches and writes its [8, 1024, 2000] output slice.

v8 structure (per core, per step). The PE matmul issue rate is capped
at ~34ns per LDWEIGHTS+MATMUL pair (weight-load path), so z uses the
fewest possible pairs: ONE [128, 256] PSUM bank, 4 column groups (one
per 256-col n-quarter), F=256 matmuls -> 32 z pairs (vs 64 at F=128):
  group j at PSUM rows 32j..32j+8 accumulates u + 8 k-tile matmuls.
The post-chain is split by column parity c1 (n = 256j + 128c1 + p, so
state block g = 2j + c1 is even iff c1=0):
  ACT:  h16_c = tanh(zps[:, 128c:128c+128])     (fp16, 2 ops)
  PE:   hT_c[p, (j,b)] = h16_c.T @ sel          (sel[32j+b, 8j+b] = 1)
  DVE:  s16'_c = xh_c + hT_c ; obuf[t] = 0.5 s16'_c ; xh_c = 0.5 s16_c
The state is kept as two fp16 tiles per ping-pong parity (even/odd g),
and the next step's z runs even-k rounds first: they only need the c1=0
chain, which completes ~300ns before the c1=1 chain, hiding the
tanh/sel/add latency of the odd half under the even rounds.
Filler matmuls into a scratch PSUM bank (plus a startup burst) keep the
PE HAM clock-gate at 2.4GHz; with idle gaps the kernel runs at 1.2GHz.
Output chunks of TC steps buffered in SBUF, DMA'd per 128-row g-block.
"""

import os
import numpy as np

import concourse.bass as bass
import concourse.mybir as mybir
import concourse.tile as tile
from concourse.bass import ds
from concourse.bass_utils import run_bass_kernel_spmd

FP32 = mybir.dt.float32
FP16 = mybir.dt.float16

ALPHA = 0.5
N_CORES = 8
B, N_IN, T, N = 64, 16, 2000, 1024
TC = 100  # steps buffered per output chunk
WARMUP_MMS = 40

LAST_EXEC_NS = None
_CACHED_NC = None


def _split_excess_waits(nc, limit=1):
    """The walrus build in this container rejects instructions carrying more
    than one sem wait; hoist extra waits onto same-engine NoOps."""
    import bass_rust
    for f in nc.m.functions:
        for bb in f.blocks:
            new_insts = []
            for ins in bb.instructions:
                si = ins.sync_info
                if si is not None and si.on_wait and len(si.on_wait) > limit:
                    waits = list(si.on_wait)
                    head, tail = waits[:-limit], waits[-limit:]
                    for j, w in enumerate(head):
                        c = bass_rust.InstNoOp(name=f"{ins.name}-w{j}")
                        c.engine = ins.engine
                        c.sync_info = mybir.SyncInfo(on_wait=[w], on_update=[])
                        new_insts.append(c)
                    si.on_wait = tail
                new_insts.append(ins)
            bb.instructions = new_insts
    return nc


def _build_nc(n=N, t_total=T, tc_steps=TC, n_in=N_IN, bc=B // N_CORES):
    G = n // 128          # 8 global 128-row n-blocks (state g index)
    GQ = 4                # 4 col groups, one per 256-col n-quarter
    KT = n // 128         # 8 k-tiles of the contraction
    FQ = n // GQ          # 256 cols per group
    n_chunks = t_total // tc_steps
    KORDER = [0, 2, 4, 6, 1, 3, 5, 7]  # even-parity k rounds first

    assert G == 8 and bc == 8

    nc = bass.Bass()
    sel_dram = nc.dram_tensor("sel", [128, 32], FP16, kind="ExternalInput")
    # w[p, (k, j, c)] = 0.5 * W[128k + p, 256j + c]
    w_dram = nc.dram_tensor("w", [128, KT * n], FP16, kind="ExternalInput")
    win_dram = nc.dram_tensor("win", [n_in, n], FP16, kind="ExternalInput")
    inpT_dram = nc.dram_tensor("inpT", [n_in, t_total, bc], FP16,
                               kind="ExternalInput")
    x_dram = nc.dram_tensor("xout", [bc, n, t_total], FP32,
                            kind="ExternalOutput")
    x_dram_r = x_dram.rearrange("b (g p) t -> p g b t", p=128)

    def w_off(k, j):
        return (k * GQ + j) * FQ

    with tile.TileContext(nc) as tc:
        with (
            tc.tile_pool(name="const", bufs=1) as const_pool,
            tc.tile_pool(name="state", bufs=1) as state_pool,
            tc.tile_pool(name="work", bufs=3) as work_pool,
            tc.tile_pool(name="obuf", bufs=2) as obuf_pool,
            tc.tile_pool(name="inp", bufs=2) as inp_pool,
            tc.tile_pool(name="psum", bufs=1, space="PSUM") as psum_pool,
            tc.tile_pool(name="psumS", bufs=1, space="PSUM") as psum_static,
        ):
            w_sb = const_pool.tile([128, KT * n], FP16)
            nc.sync.dma_start(w_sb[:, :], w_dram[:, :])
            win_sb = const_pool.tile([n_in, n], FP16)
            nc.sync.dma_start(win_sb[:, :], win_dram[:, :])
            sel_sb = const_pool.tile([128, 32], FP16)
            nc.sync.dma_start(sel_sb[:, :], sel_dram[:, :])
            zero16 = const_pool.tile([128, 256], FP16)
            nc.vector.memset(zero16[:, :], 0.0)

            # 2 static psum banks (ping-pong); zero-filled once so
            # never-written partition rows stay finite zeros (tanh reads
            # all 128 rows; tanh(0)=0 keeps the sel contraction exact)
            zpsS = [psum_static.tile([128, FQ], FP32, name=f"zps_{b_}")
                    for b_ in range(2)]
            for b_ in range(2):
                nc.tensor.matmul(
                    zpsS[b_][:, :], zero16[:, 0:128], zero16[:, :],
                    start=True, stop=True, skip_group_check=True)
            # scratch bank for HAM-warming filler matmuls
            scr = psum_static.tile([128, FQ], FP32, name="scratch")

            def filler(tag, cnt):
                for i_ in range(cnt):
                    nc.tensor.matmul(
                        scr[0:8, 0:128], zero16[:, 0:8], zero16[:, 0:128],
                        start=True, stop=True, skip_group_check=True)

            filler("warm", WARMUP_MMS)

            # State: fp16 only, split by g-parity (c1): s16s[pp][c1] holds
            # blocks g = 2*gi + c1, layout [128 p, (gi in 0..3, b in 0..7)],
            # n = 128g + p.  s16 = x_t + h_t (matmul operand; the 0.5 leak
            # is folded into W host-side), xh = 0.5*s16 = x_t.
            s16s = [[state_pool.tile([128, 32], FP16, name=f"s16_{b_}_{c_}")
                     for c_ in range(2)] for b_ in range(2)]
            xhs = [state_pool.tile([128, 32], FP16, name=f"xh_{c_}")
                   for c_ in range(2)]
            for b_ in range(2):
                for c_ in range(2):
                    nc.vector.memset(s16s[b_][c_][:, :], 0.0)
            for c_ in range(2):
                nc.vector.memset(xhs[c_][:, :], 0.0)

            def chunk_body(ci):
                inp_sb = inp_pool.tile([n_in, tc_steps * bc], FP16)
                nc.sync.dma_start(
                    inp_sb[:, :], inpT_dram[:, ds(ci * tc_steps, tc_steps), :])
                obuf = obuf_pool.tile([128, G * 8 * tc_steps], FP32)
                obuf_r = obuf[:, :].rearrange(
                    "p (gi c b t) -> p gi c b t", gi=4, c=2, b=8, t=tc_steps)
                obuf_dma = obuf[:, :].rearrange(
                    "p (g b t) -> p g b t", g=G, b=8, t=tc_steps)

                def emit_u(t):
                    # u for step t opens (start=True) rows 32j..32j+8 of the
                    # bank; the z k-tile partials land on top. Measured on
                    # HW: per-group start=True does NOT wipe other groups'
                    # writes (clear is per written region).
                    zp = zpsS[t % 2]
                    for j in range(GQ):
                        nc.tensor.matmul(
                            zp[32 * j: 32 * j + 8, :],
                            inp_sb[:, t * bc: (t + 1) * bc],
                            win_sb[:, FQ * j: FQ * j + FQ],
                            start=True, stop=False,
                            skip_group_check=True,
                            tile_position=(0, 32 * j),
                        )
                    return zp

                def emit_z(zps, s16):
                    # group j covers n-quarter j; even-k rounds first so the
                    # step is gated by the (earlier) even-parity state chain
                    for k in KORDER:
                        lhs = s16[k % 2][:, 8 * (k // 2): 8 * (k // 2) + 8]
                        for j in range(GQ):
                            nc.tensor.matmul(
                                zps[32 * j: 32 * j + 8, :],
                                lhs,
                                w_sb[:, w_off(k, j): w_off(k, j) + FQ],
                                start=False, stop=(k == KT - 1),
                                skip_group_check=True,
                                tile_position=(0, 32 * j),
                            )

                def emit_tanh(zps, c1):
                    h16 = work_pool.tile([128, 128], FP16, tag=f"h16_{c1}",
                                         name=f"h16_{c1}")
                    nc.scalar.activation(
                        h16[:, :], zps[:, 128 * c1: 128 * c1 + 128],
                        mybir.ActivationFunctionType.Tanh)
                    return h16

                def emit_sel(h16, c1):
                    # hT[p, (j,b)] = tanh(z)[b, n=256j+128c1+p]: the c1-
                    # parity half of the transposed state update
                    hTp = psum_pool.tile([128, 32], FP32,
                                         tag=f"hT{c1}", name=f"hTp{c1}")
                    nc.tensor.matmul(
                        hTp[:, :], h16[:, :], sel_sb[:, :],
                        start=True, stop=True, skip_group_check=True)
                    return hTp

                def emit_update(hTp, t, c1):
                    s16_n = s16s[(t + 1) % 2][c1]
                    nc.vector.tensor_add(s16_n[:, :], xhs[c1][:, :],
                                         hTp[:, :])
                    s_r = s16_n[:, :].rearrange("p (gi b) -> p gi b",
                                                gi=4, b=8)
                    nc.vector.tensor_scalar_mul(
                        obuf_r[:, :, c1, :, t], s_r[:, :, :], ALPHA)

                def emit_xh(t, c1):
                    nc.vector.tensor_scalar_mul(
                        xhs[c1][:, :], s16s[t % 2][c1][:, :], ALPHA)

                zps_cur = emit_u(0)
                for t in range(tc_steps):
                    s16 = s16s[t % 2]
                    zps = zps_cur
                    emit_xh(t, 0)
                    emit_xh(t, 1)
                    emit_z(zps, s16)
                    h16_0 = emit_tanh(zps, 0)
                    h16_1 = emit_tanh(zps, 1)
                    # PE order: z | u | fill | sel c0 | fill | sel c1; u and
                    # fillers cover the tanh latencies
                    if t + 1 < tc_steps:
                        zps_cur = emit_u(t + 1)
                    filler(f"s{t}a", 3)
                    hTp0 = emit_sel(h16_0, 0)
                    emit_update(hTp0, t, 0)
                    filler(f"s{t}b", 3)
                    hTp1 = emit_sel(h16_1, 1)
                    emit_update(hTp1, t, 1)

                for g in range(G):
                    nc.sync.dma_start(
                        x_dram_r[:, g, :, ds(ci * tc_steps, tc_steps)],
                        obuf_dma[:, g, :, :],
                    )

            with tc.For_i(0, n_chunks, 1) as i:
                chunk_body(i)

    _split_excess_waits(nc)
    return nc


def kernel(Input, W_in, W):
    """Full inputs in, full output out. Shards batch over 8 NeuronCores."""
    global LAST_EXEC_NS, _CACHED_NC
    Input = np.ascontiguousarray(np.asarray(Input, dtype=np.float32))
    W_in = np.ascontiguousarray(np.asarray(W_in, dtype=np.float32))
    W = np.ascontiguousarray(np.asarray(W, dtype=np.float32))
    Bf, n_in, t_total = Input.shape
    n = W.shape[0]
    bc = Bf // N_CORES

    tc_steps = TC if t_total % TC == 0 else max(
        d for d in range(1, min(TC, t_total) + 1) if t_total % d == 0)
    if _CACHED_NC is None:
        _CACHED_NC = _build_nc(n=n, t_total=t_total, tc_steps=tc_steps,
                               n_in=n_in, bc=bc)
    nc = _CACHED_NC

    # leak factor folded into W: matmul operand is s = x + h = 2x, so W/2.
    # layout [p, (k, j, c)] = 0.5*W[128k+p, 256j+c]
    w_r = np.ascontiguousarray(
        (ALPHA * W).reshape(8, 128, 4, 256).transpose(1, 0, 2, 3)
        .reshape(128, 8 * n)
    ).astype(np.float16)
    win16 = W_in.astype(np.float16)
    sel = np.zeros((128, 32), dtype=np.float16)
    for j_ in range(4):
        for b_ in range(8):
            sel[32 * j_ + b_, 8 * j_ + b_] = 1.0
    in_maps = []
    for c in range(N_CORES):
        inpT = np.ascontiguousarray(
            Input[c * bc:(c + 1) * bc].transpose(1, 2, 0)).astype(np.float16)
        in_maps.append({"w": w_r, "win": win16, "inpT": inpT, "sel": sel})

    trace = bool(int(os.environ.get("ESN_TRACE", "0")))
    res = run_bass_kernel_spmd(
        nc, in_maps, core_ids=list(range(N_CORES)), trace=trace)
    LAST_EXEC_NS = res.exec_time_ns

    out = np.concatenate([res.results[c]["xout"] for c in range(N_CORES)],
                         axis=0)
    return np.ascontiguousarray(out.astype(np.float32))
